# revision 5
# baseline (speedup 1.0000x reference)
"""TRN2 Bass kernel for nn_BSLinear_71159018160311.

Computes  out = input @ W.T  with
  W = U @ diag(weight^2 * mask) @ Vh + U_additional @ Vh_additional

Sharding: data-parallel over the B*S=16384 token dim across 8 NeuronCores
(2048 tokens/core), no collectives. Each core runs the factorized form
(t = V_eff @ x, then y = U_eff @ t) as two matmul phases.

Mixed-precision rank split: the rank-r component of W contributes with
weight s_r = weight_r^2 * mask_r, and the s^2-energy of the smallest-s half
of the ranks is only ~5% of the total. Those NS=512 low-s ranks run in fp8
e4m3 with DoubleRow perf mode (2 k-tiles contracted per PE pass - 2x
matmul rate); the high-s half runs in bf16 (full PE rate, ~0.1% error).
Measured end-to-end rel err ~1.2e-2 vs the 2e-2 gate (fp8 contributes
2*beta*sqrt(E) ~ 2*2.7%*sqrt(0.047)).

Scales (all powers of two, so rescaling is exact and program immediates are
data-independent): x*32 -> fp8, V*2048 -> fp8, U_eff*4096 -> fp8, and the
phase-1 psum (scale 32*2048) is cast to the phase-2 fp8 operand t8 with a
single tensor_scalar_mul by 2^-12 (target scale 16). The bf16 branch's U is
pre-scaled by C=16*4096=2^16 so both branches accumulate in the SAME psum
group; the output copy multiplies by 2^-16.

Phase 1: bf16 half streams x k-blocks (like the fp32r baseline) with psum
k-blocking + bf16 SBUF accumulation; fp8 half keeps x8 (8MB) resident in
SBUF and accumulates the full K=4096 contraction in psum (no SBUF adds),
then one scaled cast emits t8 in the [128, 2, TC] k-pair layout DoubleRow
wants. Phase 2 accumulates bf16 rank-tiles then fp8 rank-pairs into one
psum group per 128-row dout tile, scaled-copies to SBUF and DMAs out.

When U_additional/Vh_additional are nonzero (they are zero for this
problem instance) the A=64 extra ranks join the bf16 half (zero-padded to
a full 128 tile).
"""

import functools

import numpy as np

B, S, D_IN, D_OUT, R, A = 4, 4096, 4096, 4096, 1024, 64
N_CORES = 8
T = B * S
TC = T // N_CORES  # 2048
KT = D_IN // 128  # 32 k-tiles
KP = KT // 2  # 16 k-pairs (fp8 DoubleRow)
KB = 4  # bf16 k-tiles per stream block
NB = KT // KB  # 8 blocks
NC_OUT = D_OUT // 512  # 8 chunks of 512 dout rows

# power-of-two scales (exact rescaling, data-independent immediates)
AX, AV, AU, AT = 32.0, 2048.0, 4096.0, 16.0
C_SCALE = AT * AU  # 65536
C1 = AT / (AX * AV)  # 2^-12: psum(phase1 fp8) -> t8
INV_C = 1.0 / C_SCALE  # 2^-16: psum(phase2) -> y


@functools.lru_cache(maxsize=2)
def _build(NBF, NS8):
    """NBF: bf16 rank-tiles (128 ranks each); NS8: fp8 rank-tiles (even)."""
    import concourse.bacc as bacc
    import concourse.mybir as mybir
    import concourse.tile as tile

    NP8 = NS8 // 2  # fp8 rank-pair tiles
    NF = NBF * 128
    NS = NS8 * 128
    f8 = mybir.dt.float8e4
    bf16 = mybir.dt.bfloat16
    f32 = mybir.dt.float32
    add = mybir.AluOpType.add
    DR = mybir.MatmulPerfMode.DoubleRow

    nc = bacc.Bacc(trn_type="TRN2")
    with tile.TileContext(nc) as tc:
        with tc.tile_pool(name="dram", bufs=1, space="DRAM") as dram:
            xbf = dram.tile([KT, 128, TC], bf16, kind="ExternalInput", name="xbf")
            vbf = dram.tile([KT, 128, NF], bf16, kind="ExternalInput", name="vbf")
            ubf = dram.tile([128, NBF, D_OUT], bf16, kind="ExternalInput", name="ubf")
            if NS8:
                x8d = dram.tile([KP, 128, 2, TC], f8, kind="ExternalInput", name="x8")
                v8d = dram.tile([KP, 128, 2, NS], f8, kind="ExternalInput", name="v8")
                u8d = dram.tile(
                    [128, NP8, 2, D_OUT], f8, kind="ExternalInput", name="u8"
                )
            yT = dram.tile([D_OUT, TC], f32, kind="ExternalOutput", name="yT")

            with (
                tc.tile_pool(name="tbf", bufs=NBF) as tbfpool,
                tc.tile_pool(name="t8", bufs=max(NP8, 1)) as t8pool,
                tc.tile_pool(name="x8r", bufs=max(KP, 1)) as x8pool,
                tc.tile_pool(name="v8r", bufs=max(KP, 1)) as v8pool,
                tc.tile_pool(name="u0", bufs=1) as u0pool,
                tc.tile_pool(name="ps", bufs=2, space="PSUM") as pspool,
            ):
                t_bf = [tbfpool.tile([128, TC], bf16, name="tbf") for _ in range(NBF)]
                t8 = [t8pool.tile([128, 2, TC], f8, name="t8") for _ in range(NP8)]
                x8_t = [
                    x8pool.tile([128, 2, TC], f8, name="x8r") for _ in range(KP if NS8 else 0)
                ]
                v8_t = [
                    v8pool.tile([128, 2, NS], f8, name="v8r") for _ in range(KP if NS8 else 0)
                ]
                # phase-2 chunk-0 weights: loaded in background during phase 1
                u0bf = u0pool.tile([128, NBF, 512], bf16)
                u08 = u0pool.tile([128, max(NP8, 1), 2, 512], f8)

                # ---- phase 1: bf16 half (streamed k-blocks) ----
                with (
                    tc.tile_pool(name="xk", bufs=2 * KB) as xpool,
                    tc.tile_pool(name="vk", bufs=2 * KB) as vpool,
                ):
                    for kb in range(NB):
                        xts, vts = [], []
                        for j in range(KB):
                            k = kb * KB + j
                            xt_t = xpool.tile([128, TC], bf16, name="xk")
                            nc.sync.dma_start(xt_t[:], xbf[k])
                            vt_t = vpool.tile([128, NF], bf16, name="vk")
                            nc.sync.dma_start(vt_t[:], vbf[k])
                            xts.append(xt_t)
                            vts.append(vt_t)
                        if NS8:
                            # spread the resident fp8 loads across the blocks
                            for p in range(2 * kb, 2 * kb + 2):
                                nc.sync.dma_start(x8_t[p][:], x8d[p])
                                nc.sync.dma_start(v8_t[p][:], v8d[p])
                        if kb == 0:
                            nc.sync.dma_start(u0bf[:], ubf[:, :, 0:512])
                            if NS8:
                                nc.sync.dma_start(u08[:], u8d[:, :, :, 0:512])
                        for m in range(NBF):
                            psum = pspool.tile([128, TC], f32, name="ps")
                            for j in range(KB):
                                for n in range(4):
                                    nc.tensor.matmul(
                                        psum[:, n * 512 : (n + 1) * 512],
                                        lhsT=vts[j][:, m * 128 : (m + 1) * 128],
                                        rhs=xts[j][:, n * 512 : (n + 1) * 512],
                                        start=(j == 0),
                                        stop=(j == KB - 1),
                                    )
                            dst = t_bf[m][:, :]
                            if kb == 0:
                                nc.any.tensor_copy(dst, psum[:, :])
                            else:
                                nc.any.tensor_tensor(dst, dst, psum[:, :], add)

                # ---- phase 1: fp8 half (x8 resident, full-K psum) ----
                for m8 in range(NS8):
                    psum = pspool.tile([128, TC], f32, name="ps")
                    for p in range(KP):
                        for n in range(4):
                            nc.tensor.matmul(
                                psum[:, n * 512 : (n + 1) * 512],
                                lhsT=v8_t[p][:, :, m8 * 128 : (m8 + 1) * 128],
                                rhs=x8_t[p][:, :, n * 512 : (n + 1) * 512],
                                start=(p == 0),
                                stop=(p == KP - 1),
                                perf_mode=DR,
                            )
                    nc.any.tensor_scalar_mul(
                        t8[m8 // 2][:, m8 % 2, :], psum[:, :], C1
                    )

                # ---- phase 2 ----
                with (
                    tc.tile_pool(name="ud", bufs=2) as upool,
                    tc.tile_pool(name="ysb", bufs=4) as ypool,
                ):
                    for ci in range(NC_OUT):
                        if ci == 0:
                            ubf_t, u8_t = u0bf, u08
                        else:
                            ubf_t = upool.tile([128, NBF, 512], bf16, name="ud")
                            nc.sync.dma_start(
                                ubf_t[:], ubf[:, :, ci * 512 : (ci + 1) * 512]
                            )
                            if NS8:
                                u8_t = upool.tile(
                                    [128, NP8, 2, 512], f8, name="ud"
                                )
                                nc.sync.dma_start(
                                    u8_t[:], u8d[:, :, :, ci * 512 : (ci + 1) * 512]
                                )
                        for dd in range(4):
                            psum = pspool.tile([128, TC], f32, name="ps")
                            for kt in range(NBF):
                                for n in range(4):
                                    nc.tensor.matmul(
                                        psum[:, n * 512 : (n + 1) * 512],
                                        lhsT=ubf_t[:, kt, dd * 128 : (dd + 1) * 128],
                                        rhs=t_bf[kt][:, n * 512 : (n + 1) * 512],
                                        start=(kt == 0),
                                        stop=(NS8 == 0 and kt == NBF - 1),
                                    )
                            for pt in range(NP8):
                                for n in range(4):
                                    nc.tensor.matmul(
                                        psum[:, n * 512 : (n + 1) * 512],
                                        lhsT=u8_t[:, pt, :, dd * 128 : (dd + 1) * 128],
                                        rhs=t8[pt][:, :, n * 512 : (n + 1) * 512],
                                        start=False,
                                        stop=(pt == NP8 - 1),
                                        perf_mode=DR,
                                    )
                            row = ci * 512 + dd * 128
                            for n in range(2):
                                ysb = ypool.tile([128, 1024], f32, name="ysb")
                                nc.any.tensor_scalar_mul(
                                    ysb[:], psum[:, n * 1024 : (n + 1) * 1024], INV_C
                                )
                                nc.sync.dma_start(
                                    yT[row : row + 128, n * 1024 : (n + 1) * 1024],
                                    ysb[:],
                                )
    nc.compile()
    names = {"xbf": xbf.name, "vbf": vbf.name, "ubf": ubf.name, "yT": yT.name}
    if NS8:
        names.update({"x8": x8d.name, "v8": v8d.name, "u8": u8d.name})
    return nc, names


def _select(weight, mask, U_additional, Vh_additional):
    """Pick (NBF, NS8, fp8 rank set) from the weights. Returns rank index
    arrays so prep and build agree."""
    s = (weight * weight * mask).astype(np.float32)
    order = np.argsort(s, kind="stable")
    tot = float(np.sum(s.astype(np.float64) ** 2)) or 1.0
    # largest fp8 tile count (<=4) whose s^2-energy keeps 2*beta*sqrt(E)
    # comfortably inside the 2e-2 gate
    ns8 = 0
    for k in range(2, 5, 2):  # NS8 must be even (DoubleRow rank pairs)
        nS = k * 128
        E = float(np.sum(np.sort(s.astype(np.float64) ** 2)[:nS])) / tot
        if 2.0 * 0.027 * np.sqrt(E) < 1.5e-2:
            ns8 = k
    nS = ns8 * 128
    Sidx = np.sort(order[:nS])
    Fidx = np.sort(order[nS:])
    has_add = bool(np.asarray(U_additional).any()) and bool(
        np.asarray(Vh_additional).any()
    )
    nF = R - nS + (A if has_add else 0)
    NBF = (nF + 127) // 128
    return NBF, ns8, Sidx, Fidx, has_add


def _prep_maps(input, weight, U, Vh, U_additional, Vh_additional, mask, sel):
    import ml_dtypes

    E4 = ml_dtypes.float8_e4m3
    BF = ml_dtypes.bfloat16
    NBF, NS8, Sidx, Fidx, has_add, names = sel
    NS = NS8 * 128
    NF = NBF * 128

    s = (weight * weight * mask).astype(np.float32)

    def q8(a, sc):
        return np.clip(a * sc, -240.0, 240.0).astype(E4)

    # ---- shared (per-core-identical) operands ----
    # bf16 V: [KT, 128, NF] with ranks Fidx (+ additional + zero pad)
    VF = np.zeros((NF, D_IN), np.float32)
    VF[: len(Fidx)] = Vh[Fidx]
    nadd = 0
    if has_add:
        nadd = A
        VF[len(Fidx) : len(Fidx) + A] = Vh_additional
    vbf = np.ascontiguousarray(VF.T.reshape(KT, 128, NF).astype(BF))
    # bf16 U (pre-scaled by C so both branches share one psum scale):
    UF = np.zeros((D_OUT, NF), np.float32)
    UF[:, : len(Fidx)] = U[:, Fidx] * (s[Fidx] * C_SCALE)[None, :]
    if has_add:
        UF[:, len(Fidx) : len(Fidx) + A] = U_additional * C_SCALE
    # [128, NBF, D_OUT]: lhsT[p, kt, dout] = UF[dout, kt*128+p]
    ubf = np.ascontiguousarray(
        UF.T.reshape(NBF, 128, D_OUT).transpose(1, 0, 2).astype(BF)
    )
    shared = {names["vbf"]: vbf, names["ubf"]: ubf}
    if NS8:
        VS = Vh[Sidx]
        v8 = q8(VS.T, AV).reshape(KP, 2, 128, NS).transpose(0, 2, 1, 3)
        shared[names["v8"]] = np.ascontiguousarray(v8)
        US = U[:, Sidx] * s[Sidx][None, :]
        NP8 = NS8 // 2
        u8 = q8(US.T, AU).reshape(NP8, 2, 128, D_OUT).transpose(2, 0, 1, 3)
        shared[names["u8"]] = np.ascontiguousarray(u8)

    x2 = np.asarray(input, dtype=np.float32).reshape(T, D_IN)
    in_maps = []
    for c in range(N_CORES):
        xcT = np.ascontiguousarray(x2[c * TC : (c + 1) * TC].T)
        m = dict(shared)
        m[names["xbf"]] = np.ascontiguousarray(xcT.reshape(KT, 128, TC).astype(BF))
        if NS8:
            m[names["x8"]] = np.ascontiguousarray(
                q8(xcT, AX).reshape(KP, 2, 128, TC).transpose(0, 2, 1, 3)
            )
        in_maps.append(m)
    return in_maps


def _gather(results, yname):
    out = np.empty((T, D_OUT), np.float32)
    for c in range(N_CORES):
        out[c * TC : (c + 1) * TC] = results[c][yname].T
    return out.reshape(B, S, D_OUT)


def kernel(input, weight, U, Vh, U_additional, Vh_additional, mask, **_kw):
    from concourse.bass_utils import run_bass_kernel_spmd

    input = np.asarray(input, dtype=np.float32)
    weight = np.asarray(weight, dtype=np.float32)
    U = np.asarray(U, dtype=np.float32)
    Vh = np.asarray(Vh, dtype=np.float32)
    U_additional = np.asarray(U_additional, dtype=np.float32)
    Vh_additional = np.asarray(Vh_additional, dtype=np.float32)
    mask = np.asarray(mask, dtype=np.float32)

    sel = _select(weight, mask, U_additional, Vh_additional)
    nc, names = _build(sel[0], sel[1])
    sel = sel + (names,)
    in_maps = _prep_maps(
        input, weight, U, Vh, U_additional, Vh_additional, mask, sel
    )
    res = run_bass_kernel_spmd(nc, in_maps, core_ids=list(range(N_CORES)))
    return _gather(res.results, names["yT"])
